# revision 2
# baseline (speedup 1.0000x reference)
"""Trainium2 Bass kernel for a small dense transformer block.

Module (hardcoded shapes): B=4, T=2048, D=64, H=8, FF=256.
  q/k/v: per-head full-width linears (H, D, D) + bias
  scores = q @ k.T (unscaled), causal, softmax
  out = attn @ v, concat heads -> proj (H*D -> D) + bias
  h1 = LN(x + attn_out); y = LN(h1 + relu(h1@W1+b1)@W2+b2)

Sharding: one head per core (8 heads / 8 cores). Each core computes its
head's attention and the partial projection attn_h @ (x @ Wv_h @ Wp_h);
a ReduceScatter sums partials over cores and shards tokens 8-ways for
the LN/FFN epilogue; the host concatenates the 8 output shards.

Math folding (host-side, O(weights) only):
  q'_t = Wq.T x_t + bq, k'_s = Wk.T x_s + bk
  (k'_s)·(q'_t) = [k_s;1]·[q'_t; bk·q'_t]  -> biases folded into 65-dim
  augmented weights, contraction K=65 with a ones-row appended to x.T.
  softmax rows sum to 1 => v-bias and proj bias become the constant
  C = sum_h bv_h @ Wp_h + bp, added once after the ReduceScatter.
  V'' gets a ones column so PV matmul also produces the softmax
  denominator (unnormalized accumulate, divide at the end).
"""

import numpy as np

B, T, D, H, FF = 4, 2048, 64, 8, 256
NTOK = B * T          # 8192
SHARD = NTOK // 8     # 1024
EPS = 1e-5
F32 = np.float32

_CACHE = {}


def _build_nc(single=False, reps=1, phases='ABCDE', cvar='full', f32r=True):
    import concourse.bass as bass
    import concourse.tile as tile
    from concourse import bacc, mybir

    f32 = mybir.dt.float32
    fr = mybir.dt.float32r if f32r else mybir.dt.float32
    Act = mybir.ActivationFunctionType
    Alu = mybir.AluOpType

    nc = bacc.Bacc("TRN2", target_bir_lowering=False, debug=False, num_devices=8)

    # ---- I/O ----
    x_d = nc.dram_tensor("x", [NTOK, D], fr, kind="ExternalInput")
    xs_d = nc.dram_tensor("xs", [SHARD, D], f32, kind="ExternalInput")
    wqk_d = nc.dram_tensor("wqk", [D + 1, D + 1], fr, kind="ExternalInput")
    wkk_d = nc.dram_tensor("wkk", [D + 1, D + 1], fr, kind="ExternalInput")
    wvv_d = nc.dram_tensor("wvv", [D + 1, D + 2], fr, kind="ExternalInput")
    w1a_d = nc.dram_tensor("w1a", [D + 1, FF], fr, kind="ExternalInput")
    w2_d = nc.dram_tensor("w2", [FF, D], fr, kind="ExternalInput")
    tri_d = nc.dram_tensor("tri", [128, 128], fr, kind="ExternalInput")
    ident_d = nc.dram_tensor("ident", [128, 128], f32, kind="ExternalInput")
    ones_d = nc.dram_tensor("ones", [1, NTOK], fr, kind="ExternalInput")
    # broadcast constants, pre-replicated to 128 partitions on host
    cbc_d = nc.dram_tensor("cbc", [128, D], f32, kind="ExternalInput")
    b2bc_d = nc.dram_tensor("b2bc", [128, D], f32, kind="ExternalInput")
    g1bc_d = nc.dram_tensor("g1bc", [128, D], f32, kind="ExternalInput")
    be1bc_d = nc.dram_tensor("be1bc", [128, D], f32, kind="ExternalInput")
    g2bc_d = nc.dram_tensor("g2bc", [128, D], f32, kind="ExternalInput")
    be2bc_d = nc.dram_tensor("be2bc", [128, D], f32, kind="ExternalInput")
    out_d = nc.dram_tensor("out", [SHARD, D], f32, kind="ExternalOutput")

    NCH = NTOK // 128   # 64 chunks of 128 tokens
    TB = 512            # t-block (query) width
    NTB = T // TB       # 4 t-blocks per batch elem

    with tile.TileContext(nc) as tc:
        with (
            tc.tile_pool(name="singles", bufs=1) as singles,
            tc.tile_pool(name="ld", bufs=4) as ld,
            tc.tile_pool(name="work", bufs=3) as work,
            tc.tile_pool(name="octt", bufs=2) as octt,
            tc.tile_pool(name="ep", bufs=2) as ep,
            tc.tile_pool(name="ps_s", bufs=4, space="PSUM") as ps_s,
            tc.tile_pool(name="ps_o", bufs=2, space="PSUM") as ps_o,
            tc.tile_pool(name="ps_t", bufs=2, space="PSUM") as ps_t,
            tc.tile_pool(name="dram", bufs=1, space="DRAM") as dram,
        ):
            # ---- persistent SBUF ----
            xT = singles.tile([D + 1, NTOK], fr)     # x.T with ones row
            qT = singles.tile([D + 1, NTOK], fr)     # [q'; kappa]
            kT = singles.tile([D + 1, NTOK], fr)     # [k'; 1]
            v2 = singles.tile([128, NCH, D + 2], fr)  # v'' in [s, p] chunks
            tri = singles.tile([128, 128], fr)
            ident = singles.tile([128, 128], f32)
            identr = singles.tile([128, 128], fr)
            wqk = singles.tile([D + 1, D + 1], fr)
            wkk = singles.tile([D + 1, D + 1], fr)
            wvv = singles.tile([D + 1, D + 2], fr)
            w1a = singles.tile([D + 1, FF], fr)
            w2 = singles.tile([128, 2, D], fr)
            cbc = singles.tile([128, D], f32)
            b2bc = singles.tile([128, D], f32)
            g1bc = singles.tile([128, D], f32)
            be1bc = singles.tile([128, D], f32)
            g2bc = singles.tile([128, D], f32)
            be2bc = singles.tile([128, D], f32)
            epst = singles.tile([128, 1], f32)
            h1_all = singles.tile([128, SHARD // 128, D], f32)
            h1T = singles.tile([D + 1, SHARD], fr)
            f1rT = singles.tile([128, 2, SHARD], fr)

            rs_in1 = dram.tile([NTOK // 2, D], f32)
            rs_in2 = dram.tile([NTOK // 2, D], f32)
            rs_out1 = dram.tile([SHARD // 2, D], f32)
            rs_out2 = dram.tile([SHARD // 2, D], f32)

            nc.sync.dma_start(tri[:], tri_d[:])
            nc.sync.dma_start(ident[:], ident_d[:])
            nc.sync.dma_start(identr[:], ident_d[:].bitcast(fr))
            nc.sync.dma_start(wqk[:], wqk_d[:])
            nc.sync.dma_start(wkk[:], wkk_d[:])
            nc.sync.dma_start(wvv[:], wvv_d[:])
            nc.sync.dma_start(w1a[:], w1a_d[:])
            nc.sync.dma_start(w2[:], w2_d.rearrange("(c p) d -> p c d", p=128))
            nc.sync.dma_start(cbc[:], cbc_d[:])
            nc.sync.dma_start(b2bc[:], b2bc_d[:])
            nc.sync.dma_start(g1bc[:], g1bc_d[:])
            nc.sync.dma_start(be1bc[:], be1bc_d[:])
            nc.sync.dma_start(g2bc[:], g2bc_d[:])
            nc.sync.dma_start(be2bc[:], be2bc_d[:])
            nc.vector.memset(epst[:], EPS)
            nc.sync.dma_start(xT[D : D + 1, :], ones_d[:, :])

            def _body_once():
              # ---- phase A: x -> xT (PE transpose, 4 chunks per PSUM bank) ----
              # one wide DMA for all of x (64 small DMAs serialize on HWDGE)
              xall = singles.tile([128, NCH, D], fr, tag="xall")
              nc.sync.dma_start(xall[:], x_d.rearrange("(i p) d -> p i d", p=128))
              for g in range(NCH // 4):
                tp = ps_t.tile([D, 512], fr, tag="small")
                for u in range(4):
                    i = 4 * g + u
                    nc.tensor.transpose(
                        tp[:, 128 * u : 128 * (u + 1)], xall[:, i, :], identr[:]
                    )
                nc.any.tensor_copy(xT[:D, 512 * g : 512 * (g + 1)], tp[:])

              if 'B' not in phases:
                  return
              # ---- phase B: qT, kT, v'' ----
              for dst, w in ((qT, wqk), (kT, wkk)):
                  for i in range(NTOK // 512):
                      pq = ps_o.tile([D + 1, 512], f32, tag="acc")
                      nc.tensor.matmul(
                          pq[:], lhsT=w[:],
                          rhs=xT[:, 512 * i : 512 * (i + 1)],
                          start=True, stop=True,
                      )
                      nc.any.tensor_copy(dst[:, 512 * i : 512 * (i + 1)], pq[:])
              for g in range(NCH // 4):
                  pv = ps_t.tile([128, 4, D + 2], f32, tag="small")
                  for u in range(4):
                      i = 4 * g + u
                      nc.tensor.matmul(
                          pv[:, u, :], lhsT=xT[:, 128 * i : 128 * (i + 1)],
                          rhs=wvv[:],
                          start=True, stop=True,
                      )
                  nc.any.tensor_copy(v2[:, 4 * g : 4 * (g + 1), :], pv[:])

              if 'C' not in phases:
                  return
              # ---- phase C: attention (block-causal, scores transposed) ----
              for b in range(B):
                  base = b * T
                  for j in range(NTB):
                      t0 = base + j * TB
                      nchunks = 4 * (j + 1)
                      outT = ps_o.tile([D + 1, TB], f32, tag="acc")

                      def s_off(c, j=j):
                          o = 128 * c - TB * j
                          return o if o > 0 else 0

                      exps = []
                      # stage 0 primed: emit score mm for chunk 0 first
                      sT0 = ps_s.tile([128, TB], f32, tag="sT")
                      o0 = s_off(0)
                      nc.tensor.matmul(
                          sT0[:, o0:TB],
                          lhsT=kT[:, base : base + 128],
                          rhs=qT[:, t0 + o0 : t0 + TB],
                          start=True, stop=True,
                      )
                      sTs = {0: sT0}
                      for c in range(nchunks):
                          o = s_off(c)
                          # next chunk's score matmul first: keeps PE busy
                          # while ACT runs exp on this chunk
                          if c + 1 < nchunks:
                              o2 = s_off(c + 1)
                              s0 = base + 128 * (c + 1)
                              sTn = ps_s.tile([128, TB], f32, tag="sT")
                              nc.tensor.matmul(
                                  sTn[:, o2:TB],
                                  lhsT=kT[:, s0 : s0 + 128],
                                  rhs=qT[:, t0 + o2 : t0 + TB],
                                  start=True, stop=True,
                              )
                              sTs[c + 1] = sTn
                          sT = sTs.pop(c)
                          if cvar == 'noexp_nopv':
                              continue
                          ex = work.tile([128, TB], fr, tag="exp")
                          nc.scalar.activation(ex[:, o:TB], sT[:, o:TB], Act.Exp)
                          if 128 * c - TB * j >= 0:  # diagonal chunk: mask
                              nc.vector.tensor_mul(
                                  ex[:, o : o + 128], ex[:, o : o + 128], tri[:]
                              )
                          if cvar == 'nopv':
                              continue
                          nc.tensor.matmul(
                              outT[:, o:TB],
                              lhsT=v2[:, (base // 128) + c, : D + 1],
                              rhs=ex[:, o:TB],
                              start=(c == 0), stop=(c == nchunks - 1),
                          )
                      if cvar != 'full':
                          continue
                      # drain: normalize + transpose to [t, d], ship to rs_in
                      oc = octt.tile([D + 1, TB], f32, tag="oc")
                      nc.any.tensor_copy(oc[:], outT[:])
                      part = work.tile([128, 4, D], f32, tag="part")
                      for u in range(4):
                          tp = ps_t.tile([128, D + 1], f32, tag="small")
                          nc.tensor.transpose(
                              tp[:], oc[:, 128 * u : 128 * (u + 1)], ident[: D + 1, : D + 1]
                          )
                          rec = work.tile([128, 1], f32, tag="rec")
                          nc.vector.reciprocal(rec[:], tp[:, D : D + 1])
                          nc.vector.tensor_scalar_mul(part[:, u, :], tp[:, :D], rec[:])
                      rs_ht = rs_in1 if b < 2 else rs_in2
                      th = t0 - (0 if b < 2 else NTOK // 2)
                      nc.sync.dma_start(
                          rs_ht[th : th + TB, :].rearrange("(u p) d -> p u d", p=128),
                          part[:],
                      )
                  # overlap the reduce-scatter with the remaining batches
                  if 'D' in phases and cvar == 'full' and b in (1, 3):
                      rs_o = rs_out1 if b == 1 else rs_out2
                      rs_i = rs_in1 if b == 1 else rs_in2
                      if single:
                          nc.sync.dma_start(rs_o[:], rs_i[: SHARD // 2, :])
                      else:
                          nc.gpsimd.collective_compute(
                              "ReduceScatter",
                              Alu.add,
                              replica_groups=[list(range(8))],
                              ins=[rs_i[:]],
                              outs=[rs_o[:]],
                          )

              if 'D' not in phases:
                  return

              if 'E' not in phases:
                  return
              # ---- phase E: epilogue on the 1024-token shard (vectorized) ----
              NQ = SHARD // 128  # 8 chunks, processed as [128, 8, 64] wide ops

              def _ln_wide(z, dst, g, b):
                  """dst = LN(z) * g + b over last dim; z, dst: [128, NQ, D]."""
                  mt = ep.tile([128, NQ, 1], f32, tag="mt")
                  nc.vector.tensor_reduce(mt[:], z[:], mybir.AxisListType.X, Alu.add)
                  nc.vector.tensor_scalar_mul(mt[:], mt[:], 1.0 / D)
                  nc.vector.tensor_tensor(
                      dst[:], z[:], mt.to_broadcast(z.shape), Alu.subtract)
                  sq = ep.tile([128, NQ, D], f32, tag="sq")
                  nc.vector.tensor_mul(sq[:], dst[:], dst[:])
                  vt = ep.tile([128, NQ, 1], f32, tag="vt")
                  nc.vector.tensor_reduce(vt[:], sq[:], mybir.AxisListType.X, Alu.add)
                  sd = ep.tile([128, NQ, 1], f32, tag="sd")
                  nc.scalar.activation(
                      sd[:, :, 0], vt[:, :, 0], Act.Sqrt, bias=epst[:], scale=1.0 / D)
                  rc = ep.tile([128, NQ, 1], f32, tag="rc")
                  nc.vector.reciprocal(rc[:], sd[:])
                  nc.vector.tensor_tensor(
                      dst[:], dst[:], rc.to_broadcast(z.shape), Alu.mult)
                  nc.vector.tensor_tensor(
                      dst[:], dst[:], g[:, None, :].to_broadcast(z.shape), Alu.mult)
                  nc.vector.tensor_tensor(
                      dst[:], dst[:], b[:, None, :].to_broadcast(z.shape), Alu.add)

              nc.sync.dma_start(h1T[D : D + 1, :], ones_d[:, :SHARD])
              zt = ep.tile([128, NQ, D], f32, tag="zt")
              nc.sync.dma_start(zt[:], xs_d.rearrange("(q p) d -> p q d", p=128))
              rt = ep.tile([128, NQ, D], f32, tag="rt")
              nc.sync.dma_start(
                  rt[:, : NQ // 2, :],
                  rs_out1[:].rearrange("(q p) d -> p q d", p=128))
              nc.sync.dma_start(
                  rt[:, NQ // 2 :, :],
                  rs_out2[:].rearrange("(q p) d -> p q d", p=128))
              nc.vector.tensor_add(zt[:], zt[:], rt[:])
              nc.vector.tensor_tensor(
                  zt[:], zt[:], cbc[:, None, :].to_broadcast(zt.shape), Alu.add)
              _ln_wide(zt, h1_all, g1bc, be1bc)
              for g2_ in range(NQ // 4):
                  tp = ps_t.tile([D, 512], f32, tag="small")
                  for u in range(4):
                      q = 4 * g2_ + u
                      nc.tensor.transpose(
                          tp[:, 128 * u : 128 * (u + 1)], h1_all[:, q, :], ident[:])
                  nc.any.tensor_copy(h1T[:D, 512 * g2_ : 512 * (g2_ + 1)], tp[:])
              # FFN up + relu (transposed layout)
              for fc in range(2):
                  for i in range(SHARD // 512):
                      pf = ps_o.tile([128, 512], f32, tag="acc")
                      nc.tensor.matmul(
                          pf[:],
                          lhsT=w1a[:, 128 * fc : 128 * (fc + 1)],
                          rhs=h1T[:, 512 * i : 512 * (i + 1)],
                          start=True, stop=True,
                      )
                      nc.scalar.activation(
                          f1rT[:, fc, 512 * i : 512 * (i + 1)], pf[:], Act.Relu
                      )
              # FFN down + residual + LN2
              y_all = ep.tile([128, NQ, D], f32, tag="yt")
              for q in range(NQ):
                  p2 = ps_t.tile([128, D], f32, tag="small")
                  nc.tensor.matmul(
                      p2[:], lhsT=f1rT[:, 0, 128 * q : 128 * (q + 1)],
                      rhs=w2[:, 0, :],
                      start=True, stop=False,
                  )
                  nc.tensor.matmul(
                      p2[:], lhsT=f1rT[:, 1, 128 * q : 128 * (q + 1)],
                      rhs=w2[:, 1, :],
                      start=False, stop=True,
                  )
                  nc.any.tensor_copy(y_all[:, q, :], p2[:])
              nc.vector.tensor_tensor(
                  y_all[:], y_all[:], b2bc[:, None, :].to_broadcast(y_all.shape), Alu.add)
              nc.vector.tensor_add(y_all[:], y_all[:], h1_all[:])
              o_all = ep.tile([128, NQ, D], f32, tag="ot")
              _ln_wide(y_all, o_all, g2bc, be2bc)
              nc.sync.dma_start(
                  out_d[:].rearrange("(q p) d -> p q d", p=128), o_all[:])

            for _rep in range(reps):
                _body_once()

    nc.compile()
    return nc


def _prep_inputs(inputs, Wq, bq, Wk, bk, Wv, bv, Wp, bp, W1, b1, W2, b2,
                 g1, be1, g2, be2):
    """Host-side input prep: augmented per-head weights + per-core maps."""
    x = np.ascontiguousarray(np.asarray(inputs, dtype=F32).reshape(NTOK, D))
    Wq, bq = np.asarray(Wq, F32), np.asarray(bq, F32)
    Wk, bk = np.asarray(Wk, F32), np.asarray(bk, F32)
    Wv, bv = np.asarray(Wv, F32), np.asarray(bv, F32)
    Wp, bp = np.asarray(Wp, F32), np.asarray(bp, F32)

    bc = lambda v: np.ascontiguousarray(
        np.broadcast_to(np.asarray(v, F32).reshape(1, D), (128, D))
    )
    tri = np.triu(np.ones((128, 128), F32))  # tri[i, j] = 1 iff j >= i
    ident = np.eye(128, dtype=F32)
    ones = np.ones((1, NTOK), F32)

    C = sum(
        bv[h].astype(np.float64) @ Wp[D * h : D * (h + 1)].astype(np.float64)
        for h in range(H)
    ) + bp.astype(np.float64)

    common = dict(
        x=x, tri=tri, ident=ident, ones=ones,
        w1a=np.ascontiguousarray(np.concatenate(
            [np.asarray(W1, F32), np.asarray(b1, F32).reshape(1, FF)], axis=0)),
        w2=np.ascontiguousarray(np.asarray(W2, F32)),
        cbc=bc(C.astype(F32)), b2bc=bc(b2),
        g1bc=bc(g1), be1bc=bc(be1), g2bc=bc(g2), be2bc=bc(be2),
    )

    e64 = np.zeros((D + 1, 1), F32)
    e64[D, 0] = 1.0
    in_maps = []
    for h in range(H):
        wq_aug = np.concatenate([Wq[h], bq[h].reshape(1, D)], axis=0)  # [65, 64]
        kappa = (wq_aug.astype(np.float64) @ bk[h].astype(np.float64)).astype(F32)
        wqk = np.concatenate([wq_aug, kappa.reshape(D + 1, 1)], axis=1)
        wk_aug = np.concatenate([Wk[h], bk[h].reshape(1, D)], axis=0)
        wkk = np.concatenate([wk_aug, e64], axis=1)
        wvp = (Wv[h].astype(np.float64)
               @ Wp[D * h : D * (h + 1)].astype(np.float64)).astype(F32)
        wvv = np.concatenate(
            [np.concatenate([wvp, np.zeros((1, D), F32)], axis=0), e64,
             np.zeros((D + 1, 1), F32)], axis=1)
        half = NTOK // 2
        sh = SHARD // 2
        xs_h = np.concatenate(
            [x[sh * h : sh * (h + 1)], x[half + sh * h : half + sh * (h + 1)]])
        in_maps.append(dict(
            common,
            xs=np.ascontiguousarray(xs_h),
            wqk=np.ascontiguousarray(wqk),
            wkk=np.ascontiguousarray(wkk),
            wvv=np.ascontiguousarray(wvv),
        ))
    return in_maps


def _get_nc():
    if "nc" not in _CACHE:
        _CACHE["nc"] = _build_nc()
    return _CACHE["nc"]


def _gather(results) -> np.ndarray:
    """Reassemble per-core output shards into the full [NTOK, D] output."""
    out = np.empty((NTOK, D), F32)
    half, sh = NTOK // 2, SHARD // 2
    for c in range(8):
        shard = results[c]["out"]
        out[sh * c : sh * (c + 1)] = shard[:sh]
        out[half + sh * c : half + sh * (c + 1)] = shard[sh:]
    return out


def kernel(**inputs) -> np.ndarray:
    from concourse.bass_utils import run_bass_kernel_spmd

    in_maps = _prep_inputs(**inputs)
    nc = _get_nc()
    res = run_bass_kernel_spmd(nc, in_maps, list(range(8)))
    return _gather(res.results).reshape(B, T, D)



# revision 5
# speedup vs baseline: 1.2692x; 1.2692x over previous
"""Trainium2 Bass kernel for a small dense transformer block.

Module (hardcoded shapes): B=4, T=2048, D=64, H=8, FF=256.
  q/k/v: per-head full-width linears (H, D, D) + bias
  scores = q @ k.T (unscaled), causal, softmax
  out = attn @ v, concat heads -> proj (H*D -> D) + bias
  h1 = LN(x + attn_out); y = LN(h1 + relu(h1@W1+b1)@W2+b2)

Sharding: one head per core (8 heads / 8 cores). Each core computes its
head's attention and the partial projection attn_h @ (x @ Wv_h @ Wp_h);
per-batch ReduceScatters (bf16) sum partials over cores and shard
tokens; a pipelined per-batch epilogue (LN/FFN) finishes each shard.

Math folding (host-side):
  scores[t,s] = (x_t Wq + bq)·(x_s Wk + bk). Terms depending only on t
  cancel in softmax over s, so with G = Wq Wk^T and c = Wk @ bq:
    scores'[t,s] = x_t G x_s^T + c·x_s
  -> k-side projection kG = [G x; c·x] (65 rows), q-side = raw x with a
  ones row (xT, built on host, bf16). The q projection disappears.
  softmax rows sum to 1 => v/proj biases fold to the constant
  C = sum_h bv_h @ Wp_h + bp, added as C/8 per core in the drain.
  V gets a ones column so PV also produces the softmax denominator.
  LN sign trick: dst = (mu - z)*rstd*(-g) + b so the subtract order
  matches scalar_tensor_tensor's (scalar op in0) op1 in1 form.
"""

import numpy as np

B, T, D, H, FF = 4, 2048, 64, 8, 256
NTOK = B * T          # 8192
SHARD = NTOK // 8     # 1024
TB = 512              # query block
EPS = 1e-5
F32 = np.float32

_CACHE = {}


def _build_nc(single=False):
    import concourse.bass as bass
    import concourse.tile as tile
    from concourse import bacc, mybir

    f32 = mybir.dt.float32
    bf16 = mybir.dt.bfloat16
    Act = mybir.ActivationFunctionType
    Alu = mybir.AluOpType

    nc = bacc.Bacc("TRN2", target_bir_lowering=False, debug=False, num_devices=8)

    # ---- I/O ----
    xT_d = nc.dram_tensor("xT", [D + 1, NTOK], bf16, kind="ExternalInput")
    xs_d = nc.dram_tensor("xs", [SHARD, D], f32, kind="ExternalInput")
    wkg_d = nc.dram_tensor("wkg", [D, D + 1], bf16, kind="ExternalInput")
    wvv_d = nc.dram_tensor("wvv", [D + 1, D + 1], bf16, kind="ExternalInput")
    w1a_d = nc.dram_tensor("w1a", [D + 1, FF], bf16, kind="ExternalInput")
    w2_d = nc.dram_tensor("w2", [FF, D], bf16, kind="ExternalInput")
    tri_d = nc.dram_tensor("tri", [128, 128], bf16, kind="ExternalInput")
    ident_d = nc.dram_tensor("ident", [128, 128], f32, kind="ExternalInput")
    identb_d = nc.dram_tensor("identb", [128, 128], bf16, kind="ExternalInput")
    one128_d = nc.dram_tensor("one128", [1, 128], bf16, kind="ExternalInput")
    b2r_d = nc.dram_tensor("b2r", [1, D], bf16, kind="ExternalInput")
    # broadcast constants, pre-replicated to 128 partitions on host
    c8bc_d = nc.dram_tensor("c8bc", [128, D], f32, kind="ExternalInput")
    g1bc_d = nc.dram_tensor("g1bc", [128, D], f32, kind="ExternalInput")
    be1bc_d = nc.dram_tensor("be1bc", [128, D], f32, kind="ExternalInput")
    g2bc_d = nc.dram_tensor("g2bc", [128, D], f32, kind="ExternalInput")
    be2bc_d = nc.dram_tensor("be2bc", [128, D], f32, kind="ExternalInput")
    out_d = nc.dram_tensor("out", [SHARD, D], f32, kind="ExternalOutput")

    NCHB = T // 128       # 16 key chunks per batch
    NJB = T // TB         # 4 query blocks per batch
    QS = SHARD // B       # 256 tokens per epilogue stage

    with tile.TileContext(nc) as tc:
        with (
            tc.tile_pool(name="singles", bufs=1) as singles,
            tc.tile_pool(name="work", bufs=3) as work,
            tc.tile_pool(name="drn", bufs=2) as drn,
            tc.tile_pool(name="ep", bufs=2) as ep,
            tc.tile_pool(name="scs", bufs=2, space="PSUM") as scs,
            tc.tile_pool(name="plong", bufs=2, space="PSUM") as plong,
            tc.tile_pool(name="psm", bufs=2, space="PSUM") as psm,
            tc.tile_pool(name="dram", bufs=1, space="DRAM") as dram,
        ):
            # ---- persistent SBUF ----
            xT = singles.tile([D + 1, NTOK], bf16)
            kT = singles.tile([D + 1, NTOK], bf16)
            v2 = singles.tile([128, NTOK // 128, D + 1], bf16)
            tri = singles.tile([128, 128], bf16)
            ident = singles.tile([128, 128], f32)
            identb = singles.tile([128, 128], bf16)
            one128 = singles.tile([1, 128], bf16)
            b2r = singles.tile([1, D], bf16)
            wkg = singles.tile([D, D + 1], bf16)
            wvv = singles.tile([D + 1, D + 1], bf16)
            w1a = singles.tile([D + 1, FF], bf16)
            w2 = singles.tile([128, 2, D], bf16)
            c8bc = singles.tile([128, D], f32)
            g1bc = singles.tile([128, D], f32)
            be1bc = singles.tile([128, D], f32)
            g2bc = singles.tile([128, D], f32)
            be2bc = singles.tile([128, D], f32)
            epst = singles.tile([128, 1], f32)
            xs_all = singles.tile([128, SHARD // 128, D], f32)
            h1b = singles.tile([128, SHARD // 128, D], bf16)
            h1T = singles.tile([D + 1, SHARD], bf16)

            rs_in = [dram.tile([T, D], bf16, tag=f"rs_in{b}", name=f"rs_in{b}")
                     for b in range(B)]
            rs_out = [dram.tile([SHARD // B, D], bf16, tag=f"rs_out{b}",
                                name=f"rs_out{b}") for b in range(B)]

            # weights via gpsimd SWDGE (cheap dispatch); bulk x via SP HWDGE
            nc.gpsimd.dma_start(wkg[:], wkg_d[:])
            nc.gpsimd.dma_start(wvv[:], wvv_d[:])
            nc.gpsimd.dma_start(tri[:], tri_d[:])
            nc.gpsimd.dma_start(ident[:], ident_d[:])
            nc.gpsimd.dma_start(identb[:], identb_d[:])
            nc.gpsimd.dma_start(one128[:], one128_d[:])
            nc.gpsimd.dma_start(b2r[:], b2r_d[:])
            nc.gpsimd.dma_start(w1a[:], w1a_d[:])
            nc.gpsimd.dma_start(w2[:], w2_d.rearrange("(c p) d -> p c d", p=128))
            nc.gpsimd.dma_start(c8bc[:], c8bc_d[:])
            nc.gpsimd.dma_start(g1bc[:], g1bc_d[:])
            nc.gpsimd.dma_start(be1bc[:], be1bc_d[:])
            nc.gpsimd.dma_start(g2bc[:], g2bc_d[:])
            nc.gpsimd.dma_start(be2bc[:], be2bc_d[:])
            nc.vector.memset(epst[:], EPS)
            nc.vector.memset(h1T[D : D + 1, :], 1.0)
            for b in range(B):
                nc.sync.dma_start(xT[:, T * b : T * (b + 1)],
                                  xT_d[:, T * b : T * (b + 1)])
            nc.sync.dma_start(xs_all[:], xs_d.rearrange("(q p) d -> p q d", p=128))

            def emit_kg(b, i):
                """kT[:, b*T + 512*i : +512] = (wkg.T @ xT-slice), bf16."""
                t0 = b * T + TB * i
                pk = psm.tile([D + 1, TB], f32, tag="small")
                nc.tensor.matmul(pk[:], lhsT=wkg[:],
                                 rhs=xT[:D, t0 : t0 + TB],
                                 start=True, stop=True)
                nc.vector.tensor_copy(kT[:, t0 : t0 + TB], pk[:])

            def emit_v2(b, i):
                """v2 chunks 4i..4i+3 of batch b."""
                pv = psm.tile([128, 4, D + 1], f32, tag="small")
                for u in range(4):
                    ci = 16 * b + 4 * i + u
                    nc.tensor.matmul(pv[:, u, :],
                                     lhsT=xT[:, 128 * ci : 128 * (ci + 1)],
                                     rhs=wvv[:], start=True, stop=True)
                nc.vector.tensor_copy(
                    v2[:, 16 * b + 4 * i : 16 * b + 4 * (i + 1), :], pv[:])

            def emit_jblock(b, j):
                base = b * T
                t0 = base + TB * j
                nchunks = 4 * (j + 1)
                ngroups = nchunks // 2
                outT = plong.tile([D + 1, TB], f32, tag="acc")

                def c_off(c):
                    o = 128 * c - TB * j
                    return o if o > 0 else 0

                # score matmuls for group g: chunks (2g, 2g+1)
                def emit_scores(g):
                    st = scs.tile([128, 2, TB], f32, tag="sT")
                    for u in range(2):
                        c = 2 * g + u
                        o = c_off(c)
                        s0 = base + 128 * c
                        nc.tensor.matmul(
                            st[:, u, o:TB],
                            lhsT=kT[:, s0 : s0 + 128],
                            rhs=xT[:, t0 + o : t0 + TB],
                            start=True, stop=True)
                    return st

                sts = {0: emit_scores(0)}
                for g in range(ngroups):
                    if g + 1 < ngroups:
                        sts[g + 1] = emit_scores(g + 1)
                    st = sts.pop(g)
                    om = c_off(2 * g)  # min offset of the two chunks
                    ex = work.tile([128, 2, TB], bf16, tag="exp")
                    nc.scalar.activation(ex[:, :, om:TB], st[:, :, om:TB], Act.Exp)
                    for u in range(2):
                        c = 2 * g + u
                        o = c_off(c)
                        if c >= 4 * j:  # diagonal chunk: mask its 128-col edge
                            nc.vector.tensor_mul(
                                ex[:, u, o : o + 128], ex[:, u, o : o + 128], tri[:])
                        nc.tensor.matmul(
                            outT[:, o:TB],
                            lhsT=v2[:, 16 * b + c, :],
                            rhs=ex[:, u, o:TB],
                            start=(c == 0), stop=(c == nchunks - 1))

                # drain: normalize + transpose to [t, d], ship to rs_in[b]
                oc = drn.tile([D + 1, TB], f32, tag="oc")
                nc.vector.tensor_copy(oc[:], outT[:])
                tp = psm.tile([128, 4, D + 1], f32, tag="small")
                for u in range(4):
                    nc.tensor.transpose(
                        tp[:, u, :], oc[:, 128 * u : 128 * (u + 1)],
                        ident[: D + 1, : D + 1])
                recb = drn.tile([128, 4, 1], f32, tag="rec")
                nc.vector.reciprocal_approx_fast(recb[:], tp[:, :, D : D + 1])
                part = drn.tile([128, 4, D], bf16, tag="part")
                for u in range(4):
                    nc.vector.scalar_tensor_tensor(
                        part[:, u, :], tp[:, u, :D], recb[:, u, :], c8bc[:],
                        Alu.mult, Alu.add)
                nc.sync.dma_start(
                    rs_in[b][TB * j : TB * (j + 1), :]
                    .rearrange("(u p) d -> p u d", p=128),
                    part[:])

            def emit_rs(b):
                if single:
                    nc.sync.dma_start(rs_out[b][:], rs_in[b][: SHARD // B, :])
                else:
                    nc.gpsimd.collective_compute(
                        "ReduceScatter", Alu.add,
                        replica_groups=[list(range(8))],
                        ins=[rs_in[b][:]], outs=[rs_out[b][:]])

            def emit_stage(s):
                """Epilogue for this core's 256-token shard of batch s."""
                NQ = QS // 128  # 2

                def ln(zin, dst, g, be):
                    mt = ep.tile([128, NQ, 1], f32, tag="mt")
                    nc.vector.tensor_reduce(mt[:], zin[:], mybir.AxisListType.X,
                                            Alu.add)
                    zc = ep.tile([128, NQ, D], f32, tag="zc")
                    # zc = mu - z  (sign folded into g on host)
                    nc.vector.scalar_tensor_tensor(
                        zc[:], mt.to_broadcast(zin.shape), 1.0 / D, zin[:],
                        Alu.mult, Alu.subtract)
                    sq = ep.tile([128, NQ, D], f32, tag="sq")
                    nc.vector.tensor_mul(sq[:], zc[:], zc[:])
                    vt = ep.tile([128, NQ, 1], f32, tag="vt")
                    nc.vector.tensor_reduce(vt[:], sq[:], mybir.AxisListType.X,
                                            Alu.add)
                    sd = ep.tile([128, NQ, 1], f32, tag="sd")
                    nc.scalar.activation(sd[:, :, 0], vt[:, :, 0], Act.Sqrt,
                                         bias=epst[:], scale=1.0 / D)
                    rc = ep.tile([128, NQ, 1], f32, tag="rc")
                    nc.vector.reciprocal_approx_fast(rc[:], sd[:])
                    nc.vector.tensor_tensor(
                        zc[:], zc[:], rc.to_broadcast(zc.shape), Alu.mult)
                    nc.vector.tensor_tensor(
                        zc[:], zc[:], g[:, None, :].to_broadcast(zc.shape),
                        Alu.mult)
                    nc.vector.tensor_tensor(
                        dst[:], zc[:], be[:, None, :].to_broadcast(zc.shape),
                        Alu.add)

                rtt = ep.tile([128, NQ, D], bf16, tag="rt")
                nc.sync.dma_start(
                    rtt[:], rs_out[s][:].rearrange("(q p) d -> p q d", p=128))
                zt = ep.tile([128, NQ, D], f32, tag="zt")
                nc.vector.tensor_tensor(
                    zt[:], xs_all[:, NQ * s : NQ * (s + 1), :], rtt[:], Alu.add)
                h1s = h1b[:, NQ * s : NQ * (s + 1), :]
                ln(zt, h1s, g1bc, be1bc)
                # h1T stage slice via PE transposes
                tpE = psm.tile([D, NQ, 128], bf16, tag="small")
                for q in range(NQ):
                    nc.tensor.transpose(tpE[:, q, :], h1s[:, q, :], identb[:])
                nc.vector.tensor_copy(
                    h1T[:D, QS * s : QS * (s + 1)]
                    .rearrange("p (a c) -> p a c", a=NQ), tpE[:])
                # FFN up + relu
                f1 = ep.tile([128, 2, QS], bf16, tag="f1")
                for fc in range(2):
                    up = psm.tile([128, QS], f32, tag="small")
                    nc.tensor.matmul(up[:],
                                     lhsT=w1a[:, 128 * fc : 128 * (fc + 1)],
                                     rhs=h1T[:, QS * s : QS * (s + 1)],
                                     start=True, stop=True)
                    nc.scalar.activation(f1[:, fc, :], up[:], Act.Relu)
                # FFN down into psum: b2 + h1 + relu(h1W1+b1)W2, then LN2
                dn = psm.tile([128, NQ, D], f32, tag="small")
                for q in range(NQ):
                    nc.tensor.matmul(dn[:, q, :], lhsT=one128[:], rhs=b2r[:],
                                     start=True, stop=False)
                    nc.tensor.matmul(dn[:, q, :], lhsT=identb[:],
                                     rhs=h1s[:, q, :], start=False, stop=False)
                    for fc in range(2):
                        nc.tensor.matmul(
                            dn[:, q, :],
                            lhsT=f1[:, fc, 128 * q : 128 * (q + 1)],
                            rhs=w2[:, fc, :],
                            start=False, stop=(fc == 1))
                o_st = ep.tile([128, NQ, D], f32, tag="ot")
                ln(dn, o_st, g2bc, be2bc)
                nc.sync.dma_start(
                    out_d[QS * s : QS * (s + 1), :]
                    .rearrange("(q p) d -> p q d", p=128),
                    o_st[:])

            # ---- schedule ----
            for i in range(NJB):
                emit_kg(0, i)
                emit_v2(0, i)
            for b in range(B):
                for j in range(NJB):
                    emit_jblock(b, j)
                    if b + 1 < B:
                        if j == 0:
                            emit_kg(b + 1, 0), emit_kg(b + 1, 1)
                        elif j == 1:
                            emit_kg(b + 1, 2), emit_kg(b + 1, 3)
                        elif j == 2:
                            emit_v2(b + 1, 0), emit_v2(b + 1, 1)
                        else:
                            emit_v2(b + 1, 2), emit_v2(b + 1, 3)
                    if b >= 2 and j == 1:
                        emit_stage(b - 2)  # epilogue for RS'd batch b-2
                emit_rs(b)
            emit_stage(2)
            emit_stage(3)

    nc.compile()
    return nc


def _prep_inputs(inputs, Wq, bq, Wk, bk, Wv, bv, Wp, bp, W1, b1, W2, b2,
                 g1, be1, g2, be2):
    """Host-side input prep: folded per-head weights + per-core maps."""
    import ml_dtypes

    BF16 = ml_dtypes.bfloat16
    x = np.ascontiguousarray(np.asarray(inputs, dtype=F32).reshape(NTOK, D))
    Wq, bq = np.asarray(Wq, np.float64), np.asarray(bq, np.float64)
    Wk, bk = np.asarray(Wk, np.float64), np.asarray(bk, np.float64)
    Wv, bv = np.asarray(Wv, np.float64), np.asarray(bv, np.float64)
    Wp, bp = np.asarray(Wp, np.float64), np.asarray(bp, np.float64)

    bc = lambda v: np.ascontiguousarray(
        np.broadcast_to(np.asarray(v, F32).reshape(1, D), (128, D)))
    bcb = lambda a: np.ascontiguousarray(np.asarray(a, F32).astype(BF16))
    tri = np.triu(np.ones((128, 128), F32)).astype(BF16)
    ident = np.eye(128, dtype=F32)
    identb = ident.astype(BF16)

    xTa = np.concatenate([x.T, np.ones((1, NTOK), F32)], axis=0).astype(BF16)
    xTa = np.ascontiguousarray(xTa)

    C = sum(bv[h] @ Wp[D * h : D * (h + 1)] for h in range(H)) + bp

    common = dict(
        xT=xTa, tri=tri, ident=ident, identb=identb,
        one128=np.ones((1, 128), F32).astype(BF16),
        b2r=bcb(np.asarray(b2, F32).reshape(1, D)),
        w1a=bcb(np.concatenate(
            [np.asarray(W1, F32), np.asarray(b1, F32).reshape(1, FF)], axis=0)),
        w2=bcb(np.asarray(W2, F32)),
        c8bc=bc((C / 8).astype(F32)),
        g1bc=bc(-np.asarray(g1, F32)), be1bc=bc(be1),
        g2bc=bc(-np.asarray(g2, F32)), be2bc=bc(be2),
    )

    in_maps = []
    for h in range(H):
        # kG weights: wkg[d, r<64] = (Wq Wk^T)[r, d]; wkg[:, 64] = Wk @ bq
        G = Wq[h] @ Wk[h].T
        c = Wk[h] @ bq[h]
        wkg = np.concatenate([G.T, c.reshape(D, 1)], axis=1)  # [64, 65]
        # V path: wvv[:64, :64] = Wv @ Wp_h; ones column via xT ones row
        wvp = Wv[h] @ Wp[D * h : D * (h + 1)]
        wvv = np.zeros((D + 1, D + 1), np.float64)
        wvv[:D, :D] = wvp
        wvv[D, D] = 1.0
        # this core's token shard: [2048*b + 256*h, +256) for each batch b
        xs_h = np.concatenate(
            [x[T * b + 256 * h : T * b + 256 * (h + 1)] for b in range(B)])
        in_maps.append(dict(
            common,
            xs=np.ascontiguousarray(xs_h),
            wkg=np.ascontiguousarray(wkg.astype(F32).astype(BF16)),
            wvv=np.ascontiguousarray(wvv.astype(F32).astype(BF16)),
        ))
    return in_maps


def _gather(results) -> np.ndarray:
    """Reassemble per-core output shards into the full [NTOK, D] output."""
    out = np.empty((NTOK, D), F32)
    qs = SHARD // B  # 256
    for c in range(8):
        shard = results[c]["out"]
        for b in range(B):
            out[T * b + qs * c : T * b + qs * (c + 1)] = shard[qs * b : qs * (b + 1)]
    return out


def _get_nc():
    if "nc" not in _CACHE:
        _CACHE["nc"] = _build_nc()
    return _CACHE["nc"]


def kernel(**inputs) -> np.ndarray:
    from concourse.bass_utils import run_bass_kernel_spmd

    in_maps = _prep_inputs(**inputs)
    nc = _get_nc()
    res = run_bass_kernel_spmd(nc, in_maps, list(range(8)))
    return _gather(res.results).reshape(B, T, D)
